# revision 2
# baseline (speedup 1.0000x reference)
"""AcidBaseDense Trainium2 kernel.

Math (reference, f32):
    bw   = sign(clip(w, -1, 1))                    in {-1, 0, +1}
    h    = 10^(-x);  oh = 1e-14 / h                (oh <= 1e-13 << f32 eps of h)
    r    = (h*0.1) @ bw - (oh*0.1) @ bw            == (h*0.1) @ bw  at f32 precision
    conc = |r| / 409.6
    ph   = -log10(conc)               if r >= 0
         = -log10(1e-14 / conc)       if r <  0

Kernel strategy:
  * host: pre-transpose x so the device loads x^T with n_in on partitions
    (contraction dim must sit on partitions for the PE); shard 2-way over
    batch x 4-way over n_out across the 8 cores; no collectives needed.
  * device: A^T = 0.1*10^(-x^T) via one ACT Exp pass, split into bf16
    hi + lo (Dekker split -> f32-accurate products), S = sign(w) in bf16
    (exact).  y tile = sum_k (A_hi + A_lo) @ S accumulated in PSUM f32 over
    64 bf16 matmuls.  Epilogue in log space avoids any division:
        L = ln(max(|r|, tiny));  u = -L/ln10 - (7 - log10(409.6))
        ph = 7 + sign(r) * u
"""

import os
import sys

for _p in ("/opt/trn_rl_repo", "/root/.axon_site/_ro/trn_rl_repo"):
    if os.path.isdir(_p) and _p not in sys.path:
        sys.path.insert(0, _p)

import numpy as np

BATCH = 4096
N_IN = 4096
N_OUT = 4096
B_GROUPS = 2           # batch shards
N_GROUPS = 4           # n_out shards
B_SH = BATCH // B_GROUPS      # 2048 batch rows per core
N_SH = N_OUT // N_GROUPS      # 1024 out cols per core
KT = N_IN // 128              # 32 contraction tiles
MT = B_SH // 128              # 16 batch tiles per core
NCHUNK = 2                    # two 512-wide PSUM chunks per batch tile

LN10 = float(np.log(10.0))
# ph(+) = (ln(409.6) - L) / ln10  = -L/ln10 + log10(409.6)
# ph(-) = 14 - ph(+)
# ph    = 7 + sign(r) * (ph(+) - 7) = 7 + sign(r) * (-L/ln10 + (log10(409.6) - 7))
U_SCALE = -1.0 / LN10
U_BIAS = float(np.log10(4096 * 0.1) - 7.0)

_CACHED = {}


def _build_nc():
    import concourse.bacc as bacc
    import concourse.mybir as mybir
    import concourse.tile as tile

    F32 = mybir.dt.float32
    BF16 = mybir.dt.bfloat16
    AFT = mybir.ActivationFunctionType
    ALU = mybir.AluOpType

    nc = bacc.Bacc(trn_type="TRN2")
    xt_d = nc.dram_tensor("xt", [N_IN, B_SH], F32, kind="ExternalInput")
    w_d = nc.dram_tensor("w", [N_IN, N_SH], F32, kind="ExternalInput")
    y_d = nc.dram_tensor("y", [B_SH, N_SH], F32, kind="ExternalOutput")

    with tile.TileContext(nc) as tc:
        with (
            tc.tile_pool(name="spool", bufs=1) as spool,
            tc.tile_pool(name="wstage", bufs=2) as wstage,
            tc.tile_pool(name="mpool", bufs=2) as mpool,
            tc.tile_pool(name="epool", bufs=2) as epool,
            tc.tile_pool(name="cpool", bufs=1) as cpool,
            tc.tile_pool(name="psum", bufs=4, space="PSUM") as psum,
        ):
            bias_ln10 = cpool.tile([128, 1], F32, tag="bias")
            nc.gpsimd.memset(bias_ln10[:], -LN10)

            # ---- S = sign(w) in bf16, resident; one tile per k-tile so the
            # first m-tile's matmuls start as soon as their slice is ready.
            s_tiles = []
            for kt in range(KT):
                wst = wstage.tile([128, N_SH], F32, tag="wst")
                nc.sync.dma_start(wst[:], w_d[kt * 128:(kt + 1) * 128, :])
                s_t = spool.tile([128, N_SH], BF16, tag=f"s{kt}")
                nc.scalar.activation(s_t[:], wst[:], AFT.Sign)
                s_tiles.append(s_t)

            # ---- per batch-tile pipeline
            for m in range(MT):
                xt_t = mpool.tile([128, N_IN], F32, tag="xt_t")
                src = xt_d[:, m * 128:(m + 1) * 128].rearrange(
                    "(t p) b -> p t b", p=128
                )
                dst = xt_t[:].rearrange("p (t b) -> p t b", b=128)
                # split the 2MB strided load across DMA queues
                for q in range(4):
                    nc.sync.dma_start(
                        dst[:, q * (KT // 4):(q + 1) * (KT // 4), :],
                        src[:, q * (KT // 4):(q + 1) * (KT // 4), :],
                    )

                # A^T = exp(-ln10 * x - ln10) = 0.1 * 10^-x   [128, 4096] f32
                a32 = mpool.tile([128, N_IN], F32, tag="a32")
                nc.scalar.activation(
                    a32[:], xt_t[:], AFT.Exp, bias=bias_ln10[:], scale=-LN10
                )
                a_hi = mpool.tile([128, N_IN], BF16, tag="a_hi")
                nc.vector.tensor_copy(a_hi[:], a32[:])
                a_lo = mpool.tile([128, N_IN], BF16, tag="a_lo")
                nc.vector.tensor_sub(a_lo[:], a32[:], a_hi[:])

                y_sb = epool.tile([128, N_SH], F32, tag="y_sb")
                for n in range(NCHUNK):
                    pt = psum.tile([128, 512], F32, tag="pt")
                    for kt in range(KT):
                        nc.tensor.matmul(
                            pt[:],
                            a_hi[:, kt * 128:(kt + 1) * 128],
                            s_tiles[kt][:, n * 512:(n + 1) * 512],
                            start=(kt == 0),
                            stop=False,
                        )
                    for kt in range(KT):
                        nc.tensor.matmul(
                            pt[:],
                            a_lo[:, kt * 128:(kt + 1) * 128],
                            s_tiles[kt][:, n * 512:(n + 1) * 512],
                            start=False,
                            stop=(kt == KT - 1),
                        )

                    # epilogue: ph = 7 + sign(r) * (-ln|r|/ln10 + (log10(409.6)-7))
                    tab = epool.tile([128, 512], F32, tag="tab")
                    nc.scalar.activation(tab[:], pt[:], AFT.Abs)
                    tcl = epool.tile([128, 512], F32, tag="tcl")
                    nc.vector.tensor_scalar_max(tcl[:], tab[:], 1e-30)
                    tln = epool.tile([128, 512], F32, tag="tln")
                    nc.scalar.activation(tln[:], tcl[:], AFT.Ln)
                    tu = epool.tile([128, 512], F32, tag="tu")
                    nc.scalar.activation(
                        tu[:], tln[:], AFT.Copy, bias=U_BIAS, scale=U_SCALE
                    )
                    tsg = epool.tile([128, 512], F32, tag="tsg")
                    nc.scalar.activation(tsg[:], pt[:], AFT.Sign)
                    ych = y_sb[:, n * 512:(n + 1) * 512]
                    nc.vector.tensor_mul(ych, tsg[:], tu[:])
                    nc.vector.tensor_scalar_add(ych, ych, 7.0)

                nc.sync.dma_start(y_d[m * 128:(m + 1) * 128, :], y_sb[:])

    nc.compile()
    return nc


def kernel(x: np.ndarray, w: np.ndarray) -> np.ndarray:
    from concourse.bass_utils import run_bass_kernel_spmd

    assert x.shape == (BATCH, N_IN) and w.shape == (N_IN, N_OUT)
    x = np.ascontiguousarray(x, dtype=np.float32)
    w = np.ascontiguousarray(w, dtype=np.float32)

    if "nc" not in _CACHED:
        _CACHED["nc"] = _build_nc()
    nc = _CACHED["nc"]

    in_maps = []
    for c in range(8):
        bg, ng = divmod(c, N_GROUPS)
        xt_sh = np.ascontiguousarray(x[bg * B_SH:(bg + 1) * B_SH, :].T)
        w_sh = np.ascontiguousarray(w[:, ng * N_SH:(ng + 1) * N_SH])
        in_maps.append({"xt": xt_sh, "w": w_sh})

    trace = os.environ.get("PH_KERNEL_TRACE", "") == "1"
    res = run_bass_kernel_spmd(
        nc, in_maps, core_ids=list(range(8)), trace=trace,
        **({"trace_cores": list(range(8))} if trace else {}),
    )
    if trace:
        _CACHED["last_result"] = res

    y = np.empty((BATCH, N_OUT), dtype=np.float32)
    for c, r in enumerate(res.results):
        bg, ng = divmod(c, N_GROUPS)
        y[bg * B_SH:(bg + 1) * B_SH, ng * N_SH:(ng + 1) * N_SH] = r["y"]
    return y


# revision 3
# speedup vs baseline: 1.2331x; 1.2331x over previous
"""AcidBaseDense Trainium2 kernel.

Math (reference, f32):
    bw   = sign(clip(w, -1, 1))                    in {-1, 0, +1}
    h    = 10^(-x);  oh = 1e-14 / h                (oh <= 1e-13 << f32 eps of h)
    r    = (h*0.1) @ bw - (oh*0.1) @ bw            == (h*0.1) @ bw  at f32 precision
    conc = |r| / 409.6
    ph   = -log10(conc)               if r >= 0
         = -log10(1e-14 / conc)       if r <  0

Kernel strategy:
  * host: pre-transpose x so the device loads x^T with n_in on partitions
    (contraction dim must sit on partitions for the PE); shard 2-way over
    batch x 4-way over n_out across the 8 cores; no collectives needed.
  * device: A^T = 0.1*10^(-x^T) via one ACT Exp pass.  Precision split:
      A = A_hi (fp16) + A_lo,  |A_lo| <= 2^-11 |A|
    hi pass: fp16 matmul (full rate), S = sign(w) in fp16 (exact)
    lo pass: fp8e4 DoubleRow matmul (0.5 cyc/row) on A_lo * 2^18 quantized
      to e4m3 (error 2^-4 * 2^-11 = 2^-15 per term ~ f32-grade), S in fp8.
    Accumulate each pass in its own PSUM bank; combine in the epilogue:
      r = psum_hi + 2^-18 * psum_lo.
  * epilogue in log space avoids any division:
      L = ln(max(|r|, tiny));  u = L*(-1/ln10) + (log10(409.6) - 7)
      ph = 7 + sign(r) * u
"""

import os
import sys

for _p in ("/opt/trn_rl_repo", "/root/.axon_site/_ro/trn_rl_repo"):
    if os.path.isdir(_p) and _p not in sys.path:
        sys.path.insert(0, _p)

import numpy as np

BATCH = 4096
N_IN = 4096
N_OUT = 4096
B_GROUPS = 2           # batch shards
N_GROUPS = 4           # n_out shards
B_SH = BATCH // B_GROUPS      # 2048 batch rows per core
N_SH = N_OUT // N_GROUPS      # 1024 out cols per core
KT = N_IN // 128              # 32 contraction tiles
MT = B_SH // 128              # 16 batch tiles per core
NCHUNK = 2                    # two 512-wide PSUM chunks per batch tile

LN10 = float(np.log(10.0))
U_SCALE = -1.0 / LN10
U_BIAS = float(np.log10(4096 * 0.1) - 7.0)
LO_SCALE = float(2.0 ** 18)
LO_INV = float(2.0 ** -18)

_CACHED = {}


def _build_nc():
    import concourse.bacc as bacc
    import concourse.mybir as mybir
    import concourse.tile as tile

    F32 = mybir.dt.float32
    FP16 = mybir.dt.float16
    FP8 = mybir.dt.float8e4
    BF16 = mybir.dt.bfloat16
    AFT = mybir.ActivationFunctionType

    nc = bacc.Bacc(trn_type="TRN2")
    xt_d = nc.dram_tensor("xt", [N_IN, B_SH], F32, kind="ExternalInput")
    w_d = nc.dram_tensor("w", [N_IN, N_SH], F32, kind="ExternalInput")
    y_d = nc.dram_tensor("y", [B_SH, N_SH], F32, kind="ExternalOutput")

    with tile.TileContext(nc) as tc:
        with (
            tc.tile_pool(name="spool", bufs=1) as spool,
            tc.tile_pool(name="wstage", bufs=2) as wstage,
            tc.tile_pool(name="mpool", bufs=2) as mpool,
            tc.tile_pool(name="epool", bufs=2) as epool,
            tc.tile_pool(name="cpool", bufs=1) as cpool,
            tc.tile_pool(name="ph_pool", bufs=4, space="PSUM") as ph_pool,
            tc.tile_pool(name="pl_pool", bufs=2, space="PSUM") as pl_pool,
        ):
            bias_ln10 = cpool.tile([128, 1], F32, tag="bias")
            nc.gpsimd.memset(bias_ln10[:], -LN10)

            # m-tile prep: load x^T block, exp in place, split hi/lo
            def prep(m):
                a32 = mpool.tile([128, N_IN], F32, tag="a32")
                src = xt_d[:, m * 128:(m + 1) * 128].rearrange(
                    "(t p) b -> p t b", p=128
                )
                dst = a32[:].rearrange("p (t b) -> p t b", b=128)
                for q in range(4):
                    nc.sync.dma_start(
                        dst[:, q * (KT // 4):(q + 1) * (KT // 4), :],
                        src[:, q * (KT // 4):(q + 1) * (KT // 4), :],
                    )
                # A^T = exp(-ln10 * x - ln10) = 0.1 * 10^-x, in place
                nc.scalar.activation(
                    a32[:], a32[:], AFT.Exp, bias=bias_ln10[:], scale=-LN10
                )
                a_hi = mpool.tile([128, N_IN], FP16, tag="a_hi")
                nc.vector.tensor_copy(a_hi[:], a32[:])
                a_lo = mpool.tile([128, N_IN], BF16, tag="a_lo")
                nc.vector.tensor_sub(a_lo[:], a32[:], a_hi[:])
                a_lo8 = mpool.tile([128, N_IN], FP8, tag="a_lo8")
                nc.vector.tensor_scalar_mul(a_lo8[:], a_lo[:], LO_SCALE)
                return a_hi, a_lo8

            # ---- emission order tuned for startup:
            # xt(m=0) DMA + exp first, then S production (ACT sign -> fp16,
            # DVE copy -> fp8), then the matmul pipeline.
            a_hi0, a_lo80 = prep(0)

            s16 = spool.tile([128, KT, N_SH], FP16, tag="s16")
            s8 = spool.tile([128, KT, N_SH], FP8, tag="s8")
            for kt in range(KT):
                wst = wstage.tile([128, N_SH], F32, tag="wst")
                nc.sync.dma_start(wst[:], w_d[kt * 128:(kt + 1) * 128, :])
                nc.scalar.activation(s16[:, kt, :], wst[:], AFT.Sign)
                nc.vector.tensor_copy(s8[:, kt, :], s16[:, kt, :])

            def hi_chains(m, a_hi):
                pts = []
                for n in range(NCHUNK):
                    pt = ph_pool.tile([128, 512], F32, tag="ph")
                    for kt in range(KT):
                        nc.tensor.matmul(
                            pt[:],
                            a_hi[:, kt * 128:(kt + 1) * 128],
                            s16[:, kt, n * 512:(n + 1) * 512],
                            start=(kt == 0),
                            stop=(kt == KT - 1),
                        )
                    pts.append(pt)
                return pts

            def lo_chain(a_lo8, n):
                pl = pl_pool.tile([128, 512], F32, tag="pl")
                for t in range(KT // 2):
                    lhs = a_lo8[:, 2 * t * 128:(2 * t + 2) * 128].rearrange(
                        "p (j m) -> p j m", j=2
                    )
                    rhs = s8[:, 2 * t:2 * t + 2, n * 512:(n + 1) * 512]
                    nc.tensor.matmul(
                        pl[:], lhs, rhs,
                        start=(t == 0), stop=(t == KT // 2 - 1),
                        perf_mode=mybir.MatmulPerfMode.DoubleRow,
                    )
                return pl

            def lo_and_epilogue(m, phi, a_lo8):
                y_sb = epool.tile([128, N_SH], F32, tag="y_sb")
                for n in range(NCHUNK):
                    pl = lo_chain(a_lo8, n)
                    # r = psum_hi + 2^-18 * psum_lo
                    tr = epool.tile([128, 512], F32, tag="tr")
                    nc.scalar.activation(
                        tr[:], pl[:], AFT.Copy, bias=0.0, scale=LO_INV
                    )
                    nc.vector.tensor_add(tr[:], tr[:], phi[n][:])
                    # ph = 7 + sign(r) * (ln|r| * U_SCALE + U_BIAS)
                    tab = epool.tile([128, 512], F32, tag="tab")
                    nc.scalar.activation(tab[:], tr[:], AFT.Abs)
                    nc.vector.tensor_scalar_max(tab[:], tab[:], 1e-30)
                    tln = epool.tile([128, 512], F32, tag="tln")
                    nc.scalar.activation(tln[:], tab[:], AFT.Ln)
                    nc.scalar.activation(
                        tln[:], tln[:], AFT.Copy, bias=U_BIAS, scale=U_SCALE
                    )
                    tsg = epool.tile([128, 512], F32, tag="tsg")
                    nc.scalar.activation(tsg[:], tr[:], AFT.Sign)
                    ych = y_sb[:, n * 512:(n + 1) * 512]
                    nc.vector.tensor_mul(ych, tsg[:], tln[:])
                    nc.vector.tensor_scalar_add(ych, ych, 7.0)
                nc.sync.dma_start(y_d[m * 128:(m + 1) * 128, :], y_sb[:])

            # ---- pipeline: lo-chains lag one m-tile behind their hi-chains
            # so the first lo matmuls never wait on full-S fp8 production.
            prev = (0, hi_chains(0, a_hi0), a_lo80)
            for m in range(1, MT):
                a_hi, a_lo8 = prep(m)
                phis = hi_chains(m, a_hi)
                lo_and_epilogue(*prev)
                prev = (m, phis, a_lo8)
            lo_and_epilogue(*prev)

    nc.compile()
    return nc


def kernel(x: np.ndarray, w: np.ndarray) -> np.ndarray:
    from concourse.bass_utils import run_bass_kernel_spmd

    assert x.shape == (BATCH, N_IN) and w.shape == (N_IN, N_OUT)
    x = np.ascontiguousarray(x, dtype=np.float32)
    w = np.ascontiguousarray(w, dtype=np.float32)

    if "nc" not in _CACHED:
        _CACHED["nc"] = _build_nc()
    nc = _CACHED["nc"]

    in_maps = []
    for c in range(8):
        bg, ng = divmod(c, N_GROUPS)
        xt_sh = np.ascontiguousarray(x[bg * B_SH:(bg + 1) * B_SH, :].T)
        w_sh = np.ascontiguousarray(w[:, ng * N_SH:(ng + 1) * N_SH])
        in_maps.append({"xt": xt_sh, "w": w_sh})

    trace = os.environ.get("PH_KERNEL_TRACE", "") == "1"
    res = run_bass_kernel_spmd(
        nc, in_maps, core_ids=list(range(8)), trace=trace,
        **({"trace_cores": list(range(8))} if trace else {}),
    )
    if trace:
        _CACHED["last_result"] = res

    y = np.empty((BATCH, N_OUT), dtype=np.float32)
    for c, r in enumerate(res.results):
        bg, ng = divmod(c, N_GROUPS)
        y[bg * B_SH:(bg + 1) * B_SH, ng * N_SH:(ng + 1) * N_SH] = r["y"]
    return y


# revision 5
# speedup vs baseline: 1.2732x; 1.0325x over previous
"""AcidBaseDense Trainium2 kernel.

Math (reference, f32):
    bw   = sign(clip(w, -1, 1))                    in {-1, 0, +1}
    h    = 10^(-x);  oh = 1e-14 / h                (oh <= 1e-13 << f32 eps of h)
    r    = (h*0.1) @ bw - (oh*0.1) @ bw            == (h*0.1) @ bw  at f32 precision
    conc = |r| / 409.6
    ph   = -log10(conc)               if r >= 0
         = -log10(1e-14 / conc)       if r <  0

Kernel strategy:
  * host: pre-transpose x so the device loads x^T with n_in on partitions
    (contraction dim must sit on partitions for the PE); shard 2-way over
    batch x 4-way over n_out across the 8 cores; no collectives needed.
  * device: A^T = 0.1*10^(-x^T) via one ACT Exp pass.  Precision split:
      A = A_hi (fp16) + A_lo,  |A_lo| <= 2^-11 |A|
    hi pass: fp16 matmul (full rate), S = sign(w) in fp16 (exact)
    lo pass: fp8e4 DoubleRow matmul (0.5 cyc/row) on A_lo * 2^18 quantized
      to e4m3 (error 2^-4 * 2^-11 = 2^-15 per term ~ f32-grade), S in fp8.
    Accumulate each pass in its own PSUM bank; combine in the epilogue:
      r = psum_hi + 2^-18 * psum_lo.
  * epilogue in log space avoids any division:
      L = ln(max(|r|, tiny));  u = L*(-1/ln10) + (log10(409.6) - 7)
      ph = 7 + sign(r) * u
"""

import os
import sys

for _p in ("/opt/trn_rl_repo", "/root/.axon_site/_ro/trn_rl_repo"):
    if os.path.isdir(_p) and _p not in sys.path:
        sys.path.insert(0, _p)

import numpy as np

BATCH = 4096
N_IN = 4096
N_OUT = 4096
B_GROUPS = 2           # batch shards
N_GROUPS = 4           # n_out shards
B_SH = BATCH // B_GROUPS      # 2048 batch rows per core
N_SH = N_OUT // N_GROUPS      # 1024 out cols per core
KT = N_IN // 128              # 32 contraction tiles
MT = B_SH // 128              # 16 batch tiles per core
NCHUNK = 2                    # two 512-wide PSUM chunks per batch tile

LN10 = float(np.log(10.0))
U_SCALE = -1.0 / LN10
U_BIAS = float(np.log10(4096 * 0.1) - 7.0)
LO_SCALE = float(2.0 ** 18)
LO_INV = float(2.0 ** -18)

_CACHED = {}


def _build_nc():
    import concourse.bacc as bacc
    import concourse.mybir as mybir
    import concourse.tile as tile

    F32 = mybir.dt.float32
    FP16 = mybir.dt.float16
    FP8 = mybir.dt.float8e4
    BF16 = mybir.dt.bfloat16
    AFT = mybir.ActivationFunctionType

    nc = bacc.Bacc(trn_type="TRN2")
    xt_d = nc.dram_tensor("xt", [N_IN, B_SH], F32, kind="ExternalInput")
    w_d = nc.dram_tensor("w", [N_IN, N_SH], F32, kind="ExternalInput")
    y_d = nc.dram_tensor("y", [B_SH, N_SH], F32, kind="ExternalOutput")

    with tile.TileContext(nc) as tc:
        with (
            tc.tile_pool(name="spool", bufs=1) as spool,
            tc.tile_pool(name="wstage", bufs=2) as wstage,
            tc.tile_pool(name="mpool", bufs=2) as mpool,
            tc.tile_pool(name="epool", bufs=2) as epool,
            tc.tile_pool(name="cpool", bufs=1) as cpool,
            tc.tile_pool(name="ph_pool", bufs=4, space="PSUM") as ph_pool,
            tc.tile_pool(name="pl_pool", bufs=2, space="PSUM") as pl_pool,
        ):
            bias_ln10 = cpool.tile([128, 1], F32, tag="bias")
            nc.gpsimd.memset(bias_ln10[:], -LN10)

            # m-tile prep: load x^T block, exp in place, split hi/lo
            def prep(m):
                a32 = mpool.tile([128, N_IN], F32, tag="a32")
                src = xt_d[:, m * 128:(m + 1) * 128].rearrange(
                    "(t p) b -> p t b", p=128
                )
                dst = a32[:].rearrange("p (t b) -> p t b", b=128)
                for q in range(4):
                    nc.sync.dma_start(
                        dst[:, q * (KT // 4):(q + 1) * (KT // 4), :],
                        src[:, q * (KT // 4):(q + 1) * (KT // 4), :],
                    )
                # A^T = exp(-ln10 * x - ln10) = 0.1 * 10^-x, in place
                nc.scalar.activation(
                    a32[:], a32[:], AFT.Exp, bias=bias_ln10[:], scale=-LN10
                )
                a_hi = mpool.tile([128, N_IN], FP16, tag="a_hi")
                nc.vector.tensor_copy(a_hi[:], a32[:])
                a_lo = mpool.tile([128, N_IN], BF16, tag="a_lo")
                nc.vector.tensor_sub(a_lo[:], a32[:], a_hi[:])
                a_lo8 = mpool.tile([128, N_IN], FP8, tag="a_lo8")
                nc.vector.tensor_scalar_mul(a_lo8[:], a_lo[:], LO_SCALE)
                s8_copies(m)
                return a_hi, a_lo8

            s16 = spool.tile([128, KT, N_SH], FP16, tag="s16")
            s8 = spool.tile([128, KT, N_SH], FP8, tag="s8")

            def s8_copies(m):
                # prep(0) is emitted before the sign ops, so start at m=1
                if 1 <= m <= 8:
                    for kt in range(4 * (m - 1), 4 * m - 4 + 4):
                        nc.vector.tensor_copy(s8[:, kt, :], s16[:, kt, :])

            # ---- emission order tuned for startup:
            # xt(m=0) DMA + exp first, then S production (ACT sign -> fp16),
            # then the matmul pipeline.  s8 copies are spread across the
            # first 8 preps so DVE never starves the PE at startup.
            a_hi0, a_lo80 = prep(0)

            for kt in range(KT):
                wst = wstage.tile([128, N_SH], F32, tag="wst")
                nc.sync.dma_start(wst[:], w_d[kt * 128:(kt + 1) * 128, :])
                nc.scalar.activation(s16[:, kt, :], wst[:], AFT.Sign)

            def hi_chains(m, a_hi):
                pts = []
                for n in range(NCHUNK):
                    pt = ph_pool.tile([128, 512], F32, tag="ph")
                    for kt in range(KT):
                        nc.tensor.matmul(
                            pt[:],
                            a_hi[:, kt * 128:(kt + 1) * 128],
                            s16[:, kt, n * 512:(n + 1) * 512],
                            start=(kt == 0),
                            stop=(kt == KT - 1),
                        )
                    pts.append(pt)
                return pts

            def lo_chain(a_lo8, n):
                pl = pl_pool.tile([128, 512], F32, tag="pl")
                for t in range(KT // 2):
                    lhs = a_lo8[:, 2 * t * 128:(2 * t + 2) * 128].rearrange(
                        "p (j m) -> p j m", j=2
                    )
                    rhs = s8[:, 2 * t:2 * t + 2, n * 512:(n + 1) * 512]
                    nc.tensor.matmul(
                        pl[:], lhs, rhs,
                        start=(t == 0), stop=(t == KT // 2 - 1),
                        perf_mode=mybir.MatmulPerfMode.DoubleRow,
                    )
                return pl

            def lo_and_epilogue(m, phi, a_lo8):
                y_sb = epool.tile([128, N_SH], F32, tag="y_sb")
                for n in range(NCHUNK):
                    pl = lo_chain(a_lo8, n)
                    # r = psum_hi + 2^-18 * psum_lo
                    tr = epool.tile([128, 512], F32, tag="tr")
                    nc.scalar.activation(
                        tr[:], pl[:], AFT.Copy, bias=0.0, scale=LO_INV
                    )
                    nc.vector.tensor_add(tr[:], tr[:], phi[n][:])
                    # ph = 7 + sign(r) * (ln|r| * U_SCALE + U_BIAS)
                    tab = epool.tile([128, 512], F32, tag="tab")
                    nc.scalar.activation(tab[:], tr[:], AFT.Abs)
                    nc.vector.tensor_scalar_max(tab[:], tab[:], 1e-30)
                    tln = epool.tile([128, 512], F32, tag="tln")
                    nc.scalar.activation(tln[:], tab[:], AFT.Ln)
                    nc.scalar.activation(
                        tln[:], tln[:], AFT.Copy, bias=U_BIAS, scale=U_SCALE
                    )
                    tsg = epool.tile([128, 512], F32, tag="tsg")
                    nc.scalar.activation(tsg[:], tr[:], AFT.Sign)
                    ych = y_sb[:, n * 512:(n + 1) * 512]
                    nc.vector.tensor_mul(ych, tsg[:], tln[:])
                    nc.vector.tensor_scalar_add(ych, ych, 7.0)
                nc.gpsimd.dma_start(y_d[m * 128:(m + 1) * 128, :], y_sb[:])

            # ---- pipeline: lo-chains lag one m-tile behind their
            # hi-chains for the first tiles (so early lo matmuls never wait
            # on S_fp8 production), then run in line to shorten the tail.
            LAG_UNTIL = 3
            prev = (0, hi_chains(0, a_hi0), a_lo80)
            for m in range(1, MT):
                a_hi, a_lo8 = prep(m)
                phis = hi_chains(m, a_hi)
                if prev is not None:
                    lo_and_epilogue(*prev)
                if m >= LAG_UNTIL:
                    lo_and_epilogue(m, phis, a_lo8)
                    prev = None
                else:
                    prev = (m, phis, a_lo8)
            if prev is not None:
                lo_and_epilogue(*prev)

    nc.compile()
    return nc


def kernel(x: np.ndarray, w: np.ndarray) -> np.ndarray:
    from concourse.bass_utils import run_bass_kernel_spmd

    assert x.shape == (BATCH, N_IN) and w.shape == (N_IN, N_OUT)
    x = np.ascontiguousarray(x, dtype=np.float32)
    w = np.ascontiguousarray(w, dtype=np.float32)

    if "nc" not in _CACHED:
        _CACHED["nc"] = _build_nc()
    nc = _CACHED["nc"]

    in_maps = []
    for c in range(8):
        bg, ng = divmod(c, N_GROUPS)
        xt_sh = np.ascontiguousarray(x[bg * B_SH:(bg + 1) * B_SH, :].T)
        w_sh = np.ascontiguousarray(w[:, ng * N_SH:(ng + 1) * N_SH])
        in_maps.append({"xt": xt_sh, "w": w_sh})

    trace = os.environ.get("PH_KERNEL_TRACE", "") == "1"
    res = run_bass_kernel_spmd(
        nc, in_maps, core_ids=list(range(8)), trace=trace,
        **({"trace_cores": list(range(8))} if trace else {}),
    )
    if trace:
        _CACHED["last_result"] = res

    y = np.empty((BATCH, N_OUT), dtype=np.float32)
    for c, r in enumerate(res.results):
        bg, ng = divmod(c, N_GROUPS)
        y[bg * B_SH:(bg + 1) * B_SH, ng * N_SH:(ng + 1) * N_SH] = r["y"]
    return y
